# revision 19
# baseline (speedup 1.0000x reference)
"""Trainium2 Bass kernel for nn_BaseEncoder (ragged entity-pair encoder).

Contract: kernel(**inputs) takes the FULL unsharded inputs (numpy) and
returns the FULL output [B, Q, E, E, R] float32.

Sharding: B*Q = 8 independent (batch, query) pairs -> one per NeuronCore.
Small weights (W_head / W_tail / prototypes-for-that-b) are replicated.

Host-side prep per core (numpy):
  - gather the E*M mention rows of the per-query attention and sum over the
    M=2 mentions (the /2 and /NH scalings cancel in the later normalization),
  - layout At[l, (e, h)] (h innermost!) in bf16 so the device outer-products
    run with packed access patterns (DVE 2x mode),
  - entity means ent = mean_m seq[pos] and their first-half projections
    ep = ent @ W[:H] (tiny vs the device FLOPs), in both layouts,
  - prototypes for this b, reshaped/transposed to [2H, R*P],
  - tiny constant masks for the PE bias-broadcast matmuls.

Device kernel per core (bf16 compute, fp32 PSUM), 3 engines pipelined:
  ZZ[l, e, f, h] = At[l,(e,h)] * At[l,(f,h)]          (VectorE, 2x packed)
  mul[l, (e,f)] = tree-sum_h ZZ                        (VectorE + GpSimd)
  S[e,f]   = Gram over (l,h) of At                     (TensorE)
  ctx[h', ef] = sum_l seq[l,h'] * mul[l, ef]           (TensorE)
  cn = ctx * (1/S)                                     (ScalarE copy + VectorE)
  pre[h'', ef] = W[H:].T @ cn + ep-bias                (TensorE; bias via mask
                                                        matmul / DVE add)
  cand = tanh(pre)                                     (ScalarE)
  scores[ef, rp] = cand.T @ ptT                        (TensorE, [ef,rp] orient)
  out[ef, r] = max_p scores[ef, r*10+p]                (VectorE reduce)
"""

import numpy as np
import ml_dtypes

B, Q, L, H, E, M, R, P, NH = 2, 4, 1024, 768, 32, 2, 5, 10, 12
NCORES = 8
LT = L // 128          # 8 l-tiles
HT = H // 128          # 6 tiles of 128 along a hidden dim
EF = E * E             # 1024 entity pairs
RP = R * P             # 50 prototype rows
HC = EF // 2           # 512-wide ef chunk (= one PSUM bank of fp32)
EC = E // 2            # 16 e-rows per chunk

_CACHE = {}


def _build_program():
    import concourse.mybir as mybir
    import concourse.tile as tile
    from concourse import bacc

    bf16 = mybir.dt.bfloat16
    f32 = mybir.dt.float32
    nc = bacc.Bacc("TRN2", target_bir_lowering=False, debug=False,
                   num_devices=NCORES)

    at_d = nc.dram_tensor("at", [L, E * NH], bf16, kind="ExternalInput").ap()
    seq_d = nc.dram_tensor("seq", [L, H], bf16, kind="ExternalInput").ap()
    wh_d = nc.dram_tensor("wh", [2 * H, H], bf16, kind="ExternalInput").ap()
    wt_d = nc.dram_tensor("wt", [2 * H, H], bf16, kind="ExternalInput").ap()
    ptT_d = nc.dram_tensor("ptT", [2 * H, RP], bf16, kind="ExternalInput").ap()
    epT_d = nc.dram_tensor("epT", [2, E, H], bf16, kind="ExternalInput").ap()
    epo_d = nc.dram_tensor("epo", [2, H, E], bf16, kind="ExternalInput").ap()
    m16_d = nc.dram_tensor("m16", [E, HC], bf16, kind="ExternalInput").ap()
    m32_d = nc.dram_tensor("m32", [E, HC], bf16, kind="ExternalInput").ap()
    out_d = nc.dram_tensor("out", [EF, R], f32, kind="ExternalOutput").ap()

    with tile.TileContext(nc) as tc:
        _emit(tc, mybir, at_d, seq_d, wh_d, wt_d, ptT_d, epT_d, epo_d,
              m16_d, m32_d, out_d)

    nc.compile()
    return nc


def _emit(tc, mybir, at_d, seq_d, wh_d, wt_d, ptT_d, epT_d, epo_d, m16_d,
          m32_d, out_d):
    nc = tc.nc
    bf16 = mybir.dt.bfloat16
    f32 = mybir.dt.float32
    Alu = mybir.AluOpType
    Act = mybir.ActivationFunctionType
    Ax = mybir.AxisListType

    import contextlib
    ctx = contextlib.ExitStack()
    with ctx:
        const = ctx.enter_context(tc.tile_pool(name="const", bufs=1))
        big = ctx.enter_context(tc.tile_pool(name="big", bufs=1))
        zz = ctx.enter_context(tc.tile_pool(name="zz", bufs=3))
        mulp = ctx.enter_context(tc.tile_pool(name="mulp", bufs=12))
        ctxp = ctx.enter_context(tc.tile_pool(name="ctxp", bufs=1))
        candp = ctx.enter_context(tc.tile_pool(name="candp", bufs=14))
        prep = ctx.enter_context(tc.tile_pool(name="prep", bufs=3))
        psum = ctx.enter_context(tc.tile_pool(name="psum", bufs=1,
                                              space="PSUM"))

        # ---------------- input loads (SP HWDGE queue) ----------------
        at_sb = big.tile([128, LT, E, NH], bf16, tag="at_sb")
        at_r = at_d.rearrange("(t p) n -> p t n", p=128)
        for lt in range(LT):
            nc.sync.dma_start(
                out=at_sb[:, lt].rearrange("p e h -> p (e h)"),
                in_=at_r[:, lt])
        seq_sb = big.tile([128, LT, H], bf16, tag="seq_sb")
        nc.sync.dma_start(out=seq_sb,
                          in_=seq_d.rearrange("(t p) n -> p t n", p=128))
        wh_sb = big.tile([128, 2 * HT, H], bf16, tag="wh_sb")
        wt_sb = big.tile([128, 2 * HT, H], bf16, tag="wt_sb")
        wh_r = wh_d.rearrange("(t p) n -> p t n", p=128)
        wt_r = wt_d.rearrange("(t p) n -> p t n", p=128)
        epT_sb = [const.tile([E, H], bf16, tag=f"epT{w}", name=f"epT{w}")
                  for w in range(2)]
        epo_sb = [const.tile([128, HT, E], bf16, tag=f"epo{w}",
                             name=f"epo{w}") for w in range(2)]
        for w in range(2):
            nc.sync.dma_start(out=epT_sb[w], in_=epT_d[w])
            nc.sync.dma_start(
                out=epo_sb[w],
                in_=epo_d[w].rearrange("(t p) n -> p t n", p=128))
        m16_sb = const.tile([E, HC], bf16, tag="m16_sb")
        nc.sync.dma_start(out=m16_sb, in_=m16_d)
        m32_sb = const.tile([E, HC], bf16, tag="m32_sb")
        nc.sync.dma_start(out=m32_sb, in_=m32_d)
        for w_sb, w_r in ((wh_sb, wh_r), (wt_sb, wt_r)):
            nc.sync.dma_start(out=w_sb[:, HT:], in_=w_r[:, HT:])
        ptT_sb = const.tile([128, 2 * HT, RP], bf16, tag="ptT_sb")
        nc.sync.dma_start(out=ptT_sb,
                          in_=ptT_d.rearrange("(t p) n -> p t n", p=128))
        for w_sb, w_r in ((wh_sb, wh_r), (wt_sb, wt_r)):
            nc.sync.dma_start(out=w_sb[:, 0:HT], in_=w_r[:, 0:HT])

        ones_row = const.tile([1, 128], bf16, tag="ones_row")
        nc.vector.memset(ones_row, 1.0)
        recd = nc.dram_tensor("recd", [E, E], bf16).ap()

        # ------------- mul pipeline: DVE products + GpSimd/DVE tree -------
        def emit_prod(lt, c):
            """DVE products + GpSimd first tree level for (lt, c)."""
            v = at_sb[:, lt]                    # [128, E, NH]
            FW = E if c == 0 else EC
            es = c * EC
            z = zz.tile([128, EC, FW, NH], bf16, tag=f"zz{c}",
                        name=f"zz{c}_{lt}")
            nc.vector.tensor_mul(
                z,
                v[:, es:es + EC, None, :].broadcast_to([128, EC, FW, NH]),
                v[:, None, (0 if c == 0 else EC):, :].broadcast_to(
                    [128, EC, FW, NH]))
            t4 = zz.tile([128, EC, FW, 4], bf16, tag=f"t4{c}",
                         name=f"t4{c}_{lt}")
            nc.gpsimd.tensor_add(t4, z[:, :, :, 0:4], z[:, :, :, 4:8])
            return z, t4

        def emit_tree(lt, c, z, t4, mulA=None):
            """DVE rest-of-tree -> mul tile for (lt, c)."""
            FW = E if c == 0 else EC
            fs = 0 if c == 0 else EC
            nc.vector.tensor_add(t4, t4, z[:, :, :, 8:12])
            u2 = zz.tile([128, EC, FW, 2], bf16, tag=f"u2{c}", name=f"u2{c}")
            nc.vector.tensor_add(u2, t4[:, :, :, 0:2], t4[:, :, :, 2:4])
            mt = mulp.tile([128, HC], bf16, tag="mul", name=f"mul{c}_{lt}")
            m3 = mt.rearrange("p (e f) -> p e f", e=EC)
            nc.vector.tensor_add(m3[:, :, fs:], u2[:, :, :, 0],
                                 u2[:, :, :, 1])
            if c == 1:
                w = mulA.rearrange("p (e f) -> p e f", e=EC)[:, :, EC:]
                nc.gpsimd.tensor_copy(m3[:, :, :EC],
                                      w.rearrange("p a b -> p b a"))
            return mt

        # ---------------- phase A: chunk-0 mul + ctx, gram ----------------
        sg_ps = psum.tile([E, E], f32, tag="sg", bufs=1, name="sg_ps")
        ctxA_ps = [psum.tile([128, HC], f32, tag="ctx", bufs=HT,
                             name=f"ctxA{ht}") for ht in range(HT)]
        r2f = const.tile([E, E], f32, tag="r2f")
        r2b = const.tile([E, E], bf16, tag="r2b")
        # Gram first in PE order: it only gates on the at DMAs, so S is ready
        # early and the reciprocal/broadcast chain hides under phase A.
        k = 0
        for lt in range(LT):
            for h in range(NH):
                sl = at_sb[:, lt, :, h]
                nc.tensor.matmul(sg_ps, sl, sl, start=(k == 0),
                                 stop=(k == LT * NH - 1))
                k += 1
        nc.scalar.copy(r2f, sg_ps)

        def emit_ctx(ct_ps, lt, mt):
            for ht in range(HT):
                nc.tensor.matmul(ct_ps[ht],
                                 seq_sb[:, lt, ht * 128:(ht + 1) * 128],
                                 mt, start=(lt == 0), stop=(lt == LT - 1))

        rec1 = const.tile([1, EF], bf16, tag="rec1")
        mulA_t = []
        pend = None
        for lt in range(LT):
            z, t4 = emit_prod(lt, 0)
            if pend is not None:
                mt = emit_tree(lt - 1, 0, *pend)
                mulA_t.append(mt)
                emit_ctx(ctxA_ps, lt - 1, mt)
            pend = (z, t4)
            if lt == 2:
                nc.vector.reciprocal(r2f, r2f)
                nc.vector.tensor_copy(r2b, r2f)
                # SP (idle after loads): bounce [32,32] -> [1,1024] row
                nc.sync.dma_start(out=recd, in_=r2b)
                nc.sync.dma_start(
                    out=rec1, in_=recd.rearrange("a b -> (a b)")[None, :])
        mt = emit_tree(LT - 1, 0, *pend)
        mulA_t.append(mt)
        emit_ctx(ctxA_ps, LT - 1, mt)

        # recS broadcast to all 128 partitions via ones-matmul
        recS_sb = const.tile([128, EF], bf16, tag="recS_sb")
        for c in range(2):
            rb = psum.tile([128, HC], f32, tag="tail", bufs=1, name="recB")
            nc.tensor.matmul(rb, ones_row, rec1[:, c * HC:(c + 1) * HC],
                             start=True, stop=True)
            nc.scalar.copy(recS_sb[:, c * HC:(c + 1) * HC], rb)

        def emit_norm(c, ctx_ps):
            cn = ctxp.tile([128, HT, HC], bf16, tag=f"cn{c}", name=f"cn{c}")
            for ht in range(HT):
                nc.scalar.copy(cn[:, ht], ctx_ps[ht])
            rs = recS_sb[:, None, c * HC:(c + 1) * HC]
            nc.vector.tensor_mul(cn, cn, rs.broadcast_to([128, HT, HC]))
            return cn

        def emit_proj_pair(c, g0, cn, cand_t, tags, dve_bias=False):
            """Two proj groups (g0, g0+1) with kt-interleaved matmuls so the
            PSUM accumulation turnaround of one bank hides under the other."""
            gs = (g0, g0 + 1)
            pss = []
            for g, tag in zip(gs, tags):
                nb = HT if tag == "ctx" else 1
                pss.append(psum.tile([128, HC], f32, tag=tag, bufs=nb,
                                     name=f"proj{c}_{g}"))
            for kt in range(HT):
                for g, ps in zip(gs, pss):
                    w, ht2 = divmod(g, HT)
                    wsb = wh_sb if w == 0 else wt_sb
                    nc.tensor.matmul(
                        ps, wsb[:, HT + kt, ht2 * 128:(ht2 + 1) * 128],
                        cn[:, kt], start=(kt == 0),
                        stop=(kt == HT - 1 and dve_bias))
            for g, ps in zip(gs, pss):
                w, ht2 = divmod(g, HT)
                hs = ht2 * 128
                cd = candp.tile([128, HC], bf16, tag="cand",
                                name=f"cand{c}_{g}")
                cand_t[g] = cd
                if dve_bias:
                    # tail phase: DVE is idle -> bias-add there, tanh on
                    # ScalarE from SBUF
                    if w == 0:
                        bias = epo_sb[0][:, ht2, c * EC:(c + 1) * EC, None]
                        bias = bias.broadcast_to([128, EC, E])
                    else:
                        bias = epo_sb[1][:, ht2, None, :]
                        bias = bias.broadcast_to([128, EC, E])
                    pre = prep.tile([128, HC], f32, tag="pre",
                                    name=f"pre{c}_{g}")
                    nc.vector.tensor_add(
                        pre.rearrange("p (e f) -> p e f", e=EC),
                        ps.rearrange("p (e f) -> p e f", e=EC), bias)
                    nc.scalar.activation(cd, pre, Act.Tanh)
                else:
                    if w == 0:
                        nc.tensor.matmul(ps, epT_sb[0][:, hs:hs + 128],
                                         m16_sb, start=False, stop=True)
                    else:
                        nc.tensor.matmul(ps, epT_sb[1][:, hs:hs + 128],
                                         m32_sb, start=False, stop=True)
                    nc.scalar.activation(cd, ps, Act.Tanh)

        def emit_scores_mm(sc_ps, g, cand_t):
            # One PSUM bank holds all 4 efb accumulation regions. start=True
            # zeroes the WHOLE bank, so only the very first matmul may carry
            # it; the other chains accumulate onto the zeroed bank.
            for efb in range(4):
                nc.tensor.matmul(
                    sc_ps[:, efb],
                    cand_t[g][:, efb * 128:(efb + 1) * 128],
                    ptT_sb[:, g, :], start=(g == 0 and efb == 0),
                    stop=(g == 2 * HT - 1))

        def emit_out(c, sc_ps):
            ob = const.tile([128, 4, R], f32, tag=f"ob{c}", name=f"ob{c}")
            nc.vector.tensor_reduce(
                out=ob, in_=sc_ps.rearrange("p b (r q) -> p b r q", r=R),
                axis=Ax.X, op=Alu.max)
            nc.gpsimd.dma_start(
                out=out_d.rearrange("(t p) r -> p t r", p=128)[
                    :, c * 4:(c + 1) * 4, :],
                in_=ob)

        # ---- phase B: chunk-1 mul + ctx, interleaved with chunk-0 tail ---
        ctxB_ps = [psum.tile([128, HC], f32, tag="ctx", bufs=HT,
                             name=f"ctxB{ht}") for ht in range(HT)]
        candA = [None] * (2 * HT)
        scA = None
        cnA = None
        projA_sched = {1: 0, 2: 2, 3: 4, 4: 6, 5: 8, 6: 10}
        pend = None
        for lt in range(LT):
            z, t4 = emit_prod(lt, 1)
            if pend is not None:
                mt = emit_tree(lt - 1, 1, *pend, mulA=mulA_t[lt - 1])
                emit_ctx(ctxB_ps, lt - 1, mt)
            pend = (z, t4)
            if lt == 1:
                cnA = emit_norm(0, ctxA_ps)
            g0 = projA_sched.get(lt)
            if g0 is not None:
                emit_proj_pair(0, g0, cnA, candA, ("sg", "tail"))
                if g0 == 2 * HT - 2:
                    scA = psum.tile([128, 4, RP], f32, tag="sg", bufs=1,
                                    name="scA")
                    for gg in range(2 * HT):
                        emit_scores_mm(scA, gg, candA)
        mt = emit_tree(LT - 1, 1, *pend, mulA=mulA_t[LT - 1])
        emit_ctx(ctxB_ps, LT - 1, mt)
        cnB = emit_norm(1, ctxB_ps)
        emit_out(0, scA)

        # ---- phase C: chunk-1 tail (PE slots from freed ctx banks) ------
        # scores matmuls trail the proj pairs by one group so the PE never
        # waits on the bias-add/tanh of the pair it just produced.
        candB = [None] * (2 * HT)
        scB = psum.tile([128, 4, RP], f32, tag="tail", bufs=1, name="scB")
        for g0 in range(0, 2 * HT, 2):
            emit_proj_pair(1, g0, cnB, candB, ("ctx", "ctx"), dve_bias=True)
            if g0 >= 2:
                emit_scores_mm(scB, g0 - 2, candB)
                emit_scores_mm(scB, g0 - 1, candB)
        emit_scores_mm(scB, 2 * HT - 2, candB)
        emit_scores_mm(scB, 2 * HT - 1, candB)
        emit_out(1, scB)


def _host_prep(sequence_output, attention, W_head, W_tail, prototypes,
               mention_pos):
    """Build the per-core input maps (numpy only)."""
    bf16 = ml_dtypes.bfloat16
    seq = np.asarray(sequence_output, dtype=np.float32)
    att = np.asarray(attention, dtype=np.float32)
    whf = np.asarray(W_head, dtype=np.float32)
    wtf = np.asarray(W_tail, dtype=np.float32)
    wh, wt = whf.astype(bf16), wtf.astype(bf16)
    pro = np.asarray(prototypes, dtype=np.float32)
    pos = np.asarray(mention_pos)

    # PE bias-broadcast masks (chunk 0): m16[k, (e,f)] = (k == e) broadcasts
    # ep_head[:, e] over f; m32[j, (e,f)] = (f == j) broadcasts ep_tail[:, f].
    m16 = np.zeros((E, HC), dtype=bf16)
    for i in range(EC):
        m16[i, i * E:(i + 1) * E] = 1
    m32 = np.ascontiguousarray(
        np.tile(np.eye(E, dtype=bf16), (1, EC)).reshape(E, HC))

    in_maps = []
    for c in range(NCORES):
        b, q = divmod(c, Q)
        p_bq = pos[b, q]                       # [E, M]
        # attention gather + mention-sum: [NH, E, L] (scale dropped)
        g = att[b, q][:, p_bq, :]              # [NH, E, M, L]
        asum = g[:, :, 0, :] + g[:, :, 1, :]   # [NH, E, L]
        # At[l, (e, h)] with h innermost (packed products on device)
        at = np.ascontiguousarray(
            asum.transpose(2, 1, 0).reshape(L, E * NH)).astype(bf16)
        # entity means and their W[:H] projections (both layouts)
        ment = seq[b, q][p_bq]                 # [E, M, H]
        ent = (ment[:, 0, :] + ment[:, 1, :]) * np.float32(0.5)
        ep_h = ent @ whf[:H]                   # [E, H]
        ep_t = ent @ wtf[:H]
        epT = np.ascontiguousarray(
            np.stack([ep_h, ep_t])).astype(bf16)         # [2, E, H]
        epo = np.ascontiguousarray(
            np.stack([ep_h.T, ep_t.T])).astype(bf16)     # [2, H, E]
        ptT = np.ascontiguousarray(
            pro[b].reshape(RP, 2 * H).T).astype(bf16)
        in_maps.append({
            "at": at,
            "seq": seq[b, q].astype(bf16),
            "wh": wh,
            "wt": wt,
            "ptT": ptT,
            "epT": epT,
            "epo": epo,
            "m16": m16,
            "m32": m32,
        })
    return in_maps


def kernel(sequence_output, attention, W_head, W_tail, prototypes,
           mention_pos):
    from concourse.bass_utils import run_bass_kernel_spmd

    if "nc" not in _CACHE:
        _CACHE["nc"] = _build_program()
    nc = _CACHE["nc"]

    in_maps = _host_prep(sequence_output, attention, W_head, W_tail,
                         prototypes, mention_pos)
    res = run_bass_kernel_spmd(nc, in_maps, core_ids=list(range(NCORES)))

    out = np.empty((B, Q, E, E, R), dtype=np.float32)
    for c in range(NCORES):
        b, q = divmod(c, Q)
        out[b, q] = res.results[c]["out"].reshape(E, E, R)
    return out


# revision 24
# speedup vs baseline: 1.3310x; 1.3310x over previous
"""Trainium2 Bass kernel for nn_BaseEncoder (ragged entity-pair encoder).

Contract: kernel(**inputs) takes the FULL unsharded inputs (numpy) and
returns the FULL output [B, Q, E, E, R] float32.

Sharding: B*Q = 8 independent (batch, query) pairs -> one per NeuronCore.
Small weights (W_head / W_tail / prototypes-for-that-b) are replicated.

Host-side prep per core (numpy):
  - gather the E*M mention rows of the per-query attention and sum over the
    M=2 mentions (the /2 and /NH scalings cancel in the later normalization),
  - layout At[l, (e, h)] (h innermost!) in bf16 so the device outer-products
    run with packed access patterns (DVE 2x mode),
  - entity means ent = mean_m seq[pos] and their first-half projections
    ep = ent @ W[:H] (tiny vs the device FLOPs), in both layouts,
  - prototypes for this b, reshaped/transposed to [2H, R*P],
  - tiny constant masks for the PE bias-broadcast matmuls.

Device kernel per core (bf16 compute, fp32 PSUM), 3 engines pipelined:
  ZZ[l, e, f, h] = At[l,(e,h)] * At[l,(f,h)]          (VectorE, 2x packed)
  mul[l, (e,f)] = tree-sum_h ZZ                        (VectorE + GpSimd)
  S[e,f]   = Gram over (l,h) of At                     (TensorE)
  ctx[h', ef] = sum_l seq[l,h'] * mul[l, ef]           (TensorE)
  cn = ctx * (1/S)                                     (ScalarE copy + VectorE)
  pre[h'', ef] = W[H:].T @ cn + ep-bias                (TensorE; bias via mask
                                                        matmul / DVE add)
  cand = tanh(pre)                                     (ScalarE)
  scores[ef, rp] = cand.T @ ptT                        (TensorE, [ef,rp] orient)
  out[ef, r] = max_p scores[ef, r*10+p]                (VectorE reduce)
"""

import numpy as np
import ml_dtypes

B, Q, L, H, E, M, R, P, NH = 2, 4, 1024, 768, 32, 2, 5, 10, 12
NCORES = 8
LT = L // 128          # 8 l-tiles
HT = H // 128          # 6 tiles of 128 along a hidden dim
EF = E * E             # 1024 entity pairs
RP = R * P             # 50 prototype rows
HC = EF // 2           # 512-wide ef chunk (= one PSUM bank of fp32)
EC = E // 2            # 16 e-rows per chunk

_CACHE = {}


def _build_program():
    import concourse.mybir as mybir
    import concourse.tile as tile
    from concourse import bacc

    bf16 = mybir.dt.bfloat16
    f32 = mybir.dt.float32
    nc = bacc.Bacc("TRN2", target_bir_lowering=False, debug=False,
                   num_devices=NCORES)

    at_d = nc.dram_tensor("at", [L, E * NH], bf16, kind="ExternalInput").ap()
    seq_d = nc.dram_tensor("seq", [L, H], bf16, kind="ExternalInput").ap()
    wh_d = nc.dram_tensor("wh", [2 * H, H], bf16, kind="ExternalInput").ap()
    wt_d = nc.dram_tensor("wt", [2 * H, H], bf16, kind="ExternalInput").ap()
    ptT_d = nc.dram_tensor("ptT", [2 * H, RP], bf16, kind="ExternalInput").ap()
    epT_d = nc.dram_tensor("epT", [2, E, H], bf16, kind="ExternalInput").ap()
    epo_d = nc.dram_tensor("epo", [2, H, E], bf16, kind="ExternalInput").ap()
    m16_d = nc.dram_tensor("m16", [E, HC], bf16, kind="ExternalInput").ap()
    m32_d = nc.dram_tensor("m32", [E, HC], bf16, kind="ExternalInput").ap()
    out_d = nc.dram_tensor("out", [EF, R], f32, kind="ExternalOutput").ap()

    with tile.TileContext(nc) as tc:
        _emit(tc, mybir, at_d, seq_d, wh_d, wt_d, ptT_d, epT_d, epo_d,
              m16_d, m32_d, out_d)

    nc.compile()
    return nc


def _emit(tc, mybir, at_d, seq_d, wh_d, wt_d, ptT_d, epT_d, epo_d, m16_d,
          m32_d, out_d):
    nc = tc.nc
    bf16 = mybir.dt.bfloat16
    f32 = mybir.dt.float32
    Alu = mybir.AluOpType
    Act = mybir.ActivationFunctionType
    Ax = mybir.AxisListType

    import contextlib
    ctx = contextlib.ExitStack()
    with ctx:
        const = ctx.enter_context(tc.tile_pool(name="const", bufs=1))
        big = ctx.enter_context(tc.tile_pool(name="big", bufs=1))
        zz = ctx.enter_context(tc.tile_pool(name="zz", bufs=3))
        mulp = ctx.enter_context(tc.tile_pool(name="mulp", bufs=12))
        ctxp = ctx.enter_context(tc.tile_pool(name="ctxp", bufs=1))
        candp = ctx.enter_context(tc.tile_pool(name="candp", bufs=14))
        prep = ctx.enter_context(tc.tile_pool(name="prep", bufs=3))
        psum = ctx.enter_context(tc.tile_pool(name="psum", bufs=1,
                                              space="PSUM"))

        # ---------------- input loads (SP HWDGE queue) ----------------
        at_sb = big.tile([128, LT, E, NH], bf16, tag="at_sb")
        at_r = at_d.rearrange("(t p) n -> p t n", p=128)
        for lt in range(LT):
            nc.sync.dma_start(
                out=at_sb[:, lt].rearrange("p e h -> p (e h)"),
                in_=at_r[:, lt])
        seq_sb = big.tile([128, LT, H], bf16, tag="seq_sb")
        nc.sync.dma_start(out=seq_sb,
                          in_=seq_d.rearrange("(t p) n -> p t n", p=128))
        # only the second halves W[H:] are used on device (ep is host-side)
        wh_sb = big.tile([128, HT, H], bf16, tag="wh_sb")
        wt_sb = big.tile([128, HT, H], bf16, tag="wt_sb")
        wh_r = wh_d.rearrange("(t p) n -> p t n", p=128)
        wt_r = wt_d.rearrange("(t p) n -> p t n", p=128)
        epT_sb = [const.tile([E, H], bf16, tag=f"epT{w}", name=f"epT{w}")
                  for w in range(2)]
        epo_sb = [const.tile([128, HT, E], bf16, tag=f"epo{w}",
                             name=f"epo{w}") for w in range(2)]
        for w in range(2):
            nc.sync.dma_start(out=epT_sb[w], in_=epT_d[w])
            nc.sync.dma_start(
                out=epo_sb[w],
                in_=epo_d[w].rearrange("(t p) n -> p t n", p=128))
        m16_sb = const.tile([E, HC], bf16, tag="m16_sb")
        nc.sync.dma_start(out=m16_sb, in_=m16_d)
        m32_sb = const.tile([E, HC], bf16, tag="m32_sb")
        nc.sync.dma_start(out=m32_sb, in_=m32_d)
        for w_sb, w_r in ((wh_sb, wh_r), (wt_sb, wt_r)):
            nc.sync.dma_start(out=w_sb, in_=w_r[:, HT:])
        ptT_sb = const.tile([128, 2 * HT, RP], bf16, tag="ptT_sb")
        nc.sync.dma_start(out=ptT_sb,
                          in_=ptT_d.rearrange("(t p) n -> p t n", p=128))

        ones_row = const.tile([1, 128], bf16, tag="ones_row")
        nc.vector.memset(ones_row, 1.0)
        recd = nc.dram_tensor("recd", [E, E], bf16).ap()

        # ------------- mul pipeline: DVE products + tree (GpSimd assists
        # with the first tree level on even phase-A tiles only; it is too
        # slow (~5us/op) to carry more without stalling the DVE).
        def emit_prod(lt, c, gps=False):
            v = at_sb[:, lt]                    # [128, E, NH]
            FW = E if c == 0 else EC
            es = c * EC
            z = zz.tile([128, EC, FW, NH], bf16, tag=f"zz{c}",
                        name=f"zz{c}_{lt}")
            nc.vector.tensor_mul(
                z,
                v[:, es:es + EC, None, :].broadcast_to([128, EC, FW, NH]),
                v[:, None, (0 if c == 0 else EC):, :].broadcast_to(
                    [128, EC, FW, NH]))
            t4 = zz.tile([128, EC, FW, 4], bf16, tag=f"t4{c}",
                         name=f"t4{c}_{lt}")
            if gps:
                nc.gpsimd.tensor_add(t4, z[:, :, :, 0:4], z[:, :, :, 4:8])
            return z, t4, gps

        def emit_tree(lt, c, z, t4, gps, mulA=None):
            """DVE rest-of-tree -> mul tile for (lt, c)."""
            FW = E if c == 0 else EC
            fs = 0 if c == 0 else EC
            if not gps:
                nc.vector.tensor_add(t4, z[:, :, :, 0:4], z[:, :, :, 4:8])
            nc.vector.tensor_add(t4, t4, z[:, :, :, 8:12])
            u2 = zz.tile([128, EC, FW, 2], bf16, tag=f"u2{c}", name=f"u2{c}")
            nc.vector.tensor_add(u2, t4[:, :, :, 0:2], t4[:, :, :, 2:4])
            mt = mulp.tile([128, HC], bf16, tag="mul", name=f"mul{c}_{lt}")
            m3 = mt.rearrange("p (e f) -> p e f", e=EC)
            nc.vector.tensor_add(m3[:, :, fs:], u2[:, :, :, 0],
                                 u2[:, :, :, 1])
            if c == 1:
                w = mulA.rearrange("p (e f) -> p e f", e=EC)[:, :, EC:]
                nc.vector.tensor_copy(m3[:, :, :EC],
                                      w.rearrange("p a b -> p b a"))
            return mt

        # ---------------- phase A: chunk-0 mul + ctx, gram ----------------
        sg_ps = psum.tile([E, E], f32, tag="sg", bufs=1, name="sg_ps")
        ctxA_ps = [psum.tile([128, HC], f32, tag="ctx", bufs=HT,
                             name=f"ctxA{ht}") for ht in range(HT)]
        r2f = const.tile([E, E], f32, tag="r2f")
        r2b = const.tile([E, E], bf16, tag="r2b")
        # Gram first in PE order: it only gates on the at DMAs, so S is ready
        # early and the reciprocal/broadcast chain hides under phase A.
        k = 0
        for lt in range(LT):
            for h in range(NH):
                sl = at_sb[:, lt, :, h]
                nc.tensor.matmul(sg_ps, sl, sl, start=(k == 0),
                                 stop=(k == LT * NH - 1))
                k += 1
        nc.scalar.copy(r2f, sg_ps)

        def emit_ctx(ct_ps, lt, mt):
            for ht in range(HT):
                nc.tensor.matmul(ct_ps[ht],
                                 seq_sb[:, lt, ht * 128:(ht + 1) * 128],
                                 mt, start=(lt == 0), stop=(lt == LT - 1))

        rec1 = const.tile([1, EF], bf16, tag="rec1")
        mulA_t = []
        pend = None
        for lt in range(LT):
            z, t4, gps = emit_prod(lt, 0, gps=(lt % 2 == 0))
            if pend is not None:
                mt = emit_tree(lt - 1, 0, *pend)
                mulA_t.append(mt)
                emit_ctx(ctxA_ps, lt - 1, mt)
            pend = (z, t4, gps)
            if lt == 2:
                nc.vector.reciprocal(r2f, r2f)
                nc.vector.tensor_copy(r2b, r2f)
                # SP (idle after loads): bounce [32,32] -> [1,1024] row
                nc.sync.dma_start(out=recd, in_=r2b)
                nc.sync.dma_start(
                    out=rec1, in_=recd.rearrange("a b -> (a b)")[None, :])
        mt = emit_tree(LT - 1, 0, *pend)
        mulA_t.append(mt)
        emit_ctx(ctxA_ps, LT - 1, mt)

        # recS broadcast to all 128 partitions via ones-matmul
        recS_sb = const.tile([128, EF], bf16, tag="recS_sb")
        for c in range(2):
            rb = psum.tile([128, HC], f32, tag="tail", bufs=1, name="recB")
            nc.tensor.matmul(rb, ones_row, rec1[:, c * HC:(c + 1) * HC],
                             start=True, stop=True)
            nc.scalar.copy(recS_sb[:, c * HC:(c + 1) * HC], rb)

        def emit_norm(c, ctx_ps):
            cn = ctxp.tile([128, HT, HC], bf16, tag=f"cn{c}", name=f"cn{c}")
            for ht in range(HT):
                nc.scalar.copy(cn[:, ht], ctx_ps[ht])
            rs = recS_sb[:, None, c * HC:(c + 1) * HC]
            nc.vector.tensor_mul(cn, cn, rs.broadcast_to([128, HT, HC]))
            return cn

        def emit_proj_pair(c, g0, cn, cand_t, tags, dve_bias=False):
            """Two proj groups (g0, g0+1) with kt-interleaved matmuls so the
            PSUM accumulation turnaround of one bank hides under the other."""
            gs = (g0, g0 + 1)
            pss = []
            for g, tag in zip(gs, tags):
                nb = HT if tag == "ctx" else 1
                pss.append(psum.tile([128, HC], f32, tag=tag, bufs=nb,
                                     name=f"proj{c}_{g}"))
            for kt in range(HT):
                for g, ps in zip(gs, pss):
                    w, ht2 = divmod(g, HT)
                    wsb = wh_sb if w == 0 else wt_sb
                    nc.tensor.matmul(
                        ps, wsb[:, kt, ht2 * 128:(ht2 + 1) * 128],
                        cn[:, kt], start=(kt == 0),
                        stop=(kt == HT - 1 and dve_bias))
            for g, ps in zip(gs, pss):
                w, ht2 = divmod(g, HT)
                hs = ht2 * 128
                cd = candp.tile([128, HC], bf16, tag="cand",
                                name=f"cand{c}_{g}")
                cand_t[g] = cd
                if dve_bias:
                    # tail phase: DVE is idle -> bias-add there, tanh on
                    # ScalarE from SBUF
                    if w == 0:
                        bias = epo_sb[0][:, ht2, c * EC:(c + 1) * EC, None]
                        bias = bias.broadcast_to([128, EC, E])
                    else:
                        bias = epo_sb[1][:, ht2, None, :]
                        bias = bias.broadcast_to([128, EC, E])
                    pre = prep.tile([128, HC], f32, tag="pre",
                                    name=f"pre{c}_{g}")
                    nc.vector.tensor_add(
                        pre.rearrange("p (e f) -> p e f", e=EC),
                        ps.rearrange("p (e f) -> p e f", e=EC), bias)
                    nc.scalar.activation(cd, pre, Act.Tanh)
                else:
                    if w == 0:
                        nc.tensor.matmul(ps, epT_sb[0][:, hs:hs + 128],
                                         m16_sb, start=False, stop=True)
                    else:
                        nc.tensor.matmul(ps, epT_sb[1][:, hs:hs + 128],
                                         m32_sb, start=False, stop=True)
                    nc.scalar.activation(cd, ps, Act.Tanh)

        def emit_scores_mm(sc_ps, g, cand_t):
            # One PSUM bank holds all 4 efb accumulation regions. start=True
            # zeroes the WHOLE bank, so only the very first matmul may carry
            # it; the other chains accumulate onto the zeroed bank.
            for efb in range(4):
                nc.tensor.matmul(
                    sc_ps[:, efb],
                    cand_t[g][:, efb * 128:(efb + 1) * 128],
                    ptT_sb[:, g, :], start=(g == 0 and efb == 0),
                    stop=(g == 2 * HT - 1))

        def emit_out(c, sc_ps):
            ob = const.tile([128, 4, R], f32, tag=f"ob{c}", name=f"ob{c}")
            nc.vector.tensor_reduce(
                out=ob, in_=sc_ps.rearrange("p b (r q) -> p b r q", r=R),
                axis=Ax.X, op=Alu.max)
            nc.gpsimd.dma_start(
                out=out_d.rearrange("(t p) r -> p t r", p=128)[
                    :, c * 4:(c + 1) * 4, :],
                in_=ob)

        # ---- phase B: chunk-1 mul + ctx, interleaved with chunk-0 tail ---
        ctxB_ps = [psum.tile([128, HC], f32, tag="ctx", bufs=HT,
                             name=f"ctxB{ht}") for ht in range(HT)]
        candA = [None] * (2 * HT)
        scA = None
        cnA = None
        projA_sched = {1: 0, 2: 2, 3: 4, 4: 6, 5: 8, 6: 10}
        pend = None
        for lt in range(LT):
            z, t4, gps = emit_prod(lt, 1)
            if pend is not None:
                mt = emit_tree(lt - 1, 1, *pend, mulA=mulA_t[lt - 1])
                emit_ctx(ctxB_ps, lt - 1, mt)
            pend = (z, t4, gps)
            if lt == 1:
                cnA = emit_norm(0, ctxA_ps)
            g0 = projA_sched.get(lt)
            if g0 is not None:
                emit_proj_pair(0, g0, cnA, candA, ("sg", "tail"))
                if g0 == 2 * HT - 2:
                    scA = psum.tile([128, 4, RP], f32, tag="sg", bufs=1,
                                    name="scA")
                    for gg in range(2 * HT):
                        emit_scores_mm(scA, gg, candA)
        mt = emit_tree(LT - 1, 1, *pend, mulA=mulA_t[LT - 1])
        emit_ctx(ctxB_ps, LT - 1, mt)
        cnB = emit_norm(1, ctxB_ps)
        emit_out(0, scA)

        # ---- phase C: chunk-1 tail (PE slots from freed ctx banks) ------
        # scores matmuls trail the proj pairs by one group so the PE never
        # waits on the bias-add/tanh of the pair it just produced.
        candB = [None] * (2 * HT)
        scB = psum.tile([128, 4, RP], f32, tag="tail", bufs=1, name="scB")
        for g0 in range(0, 2 * HT, 2):
            emit_proj_pair(1, g0, cnB, candB, ("ctx", "ctx"), dve_bias=True)
            if g0 >= 2:
                emit_scores_mm(scB, g0 - 2, candB)
                emit_scores_mm(scB, g0 - 1, candB)
        emit_scores_mm(scB, 2 * HT - 2, candB)
        emit_scores_mm(scB, 2 * HT - 1, candB)
        emit_out(1, scB)


def _host_prep(sequence_output, attention, W_head, W_tail, prototypes,
               mention_pos):
    """Build the per-core input maps (numpy only)."""
    bf16 = ml_dtypes.bfloat16
    seq = np.asarray(sequence_output, dtype=np.float32)
    att = np.asarray(attention, dtype=np.float32)
    whf = np.asarray(W_head, dtype=np.float32)
    wtf = np.asarray(W_tail, dtype=np.float32)
    wh, wt = whf.astype(bf16), wtf.astype(bf16)
    pro = np.asarray(prototypes, dtype=np.float32)
    pos = np.asarray(mention_pos)

    # PE bias-broadcast masks (chunk 0): m16[k, (e,f)] = (k == e) broadcasts
    # ep_head[:, e] over f; m32[j, (e,f)] = (f == j) broadcasts ep_tail[:, f].
    m16 = np.zeros((E, HC), dtype=bf16)
    for i in range(EC):
        m16[i, i * E:(i + 1) * E] = 1
    m32 = np.ascontiguousarray(
        np.tile(np.eye(E, dtype=bf16), (1, EC)).reshape(E, HC))

    in_maps = []
    for c in range(NCORES):
        b, q = divmod(c, Q)
        p_bq = pos[b, q]                       # [E, M]
        # attention gather + mention-sum: [NH, E, L] (scale dropped)
        g = att[b, q][:, p_bq, :]              # [NH, E, M, L]
        asum = g[:, :, 0, :] + g[:, :, 1, :]   # [NH, E, L]
        # At[l, (e, h)] with h innermost (packed products on device)
        at = np.ascontiguousarray(
            asum.transpose(2, 1, 0).reshape(L, E * NH)).astype(bf16)
        # entity means and their W[:H] projections (both layouts)
        ment = seq[b, q][p_bq]                 # [E, M, H]
        ent = (ment[:, 0, :] + ment[:, 1, :]) * np.float32(0.5)
        ep_h = ent @ whf[:H]                   # [E, H]
        ep_t = ent @ wtf[:H]
        epT = np.ascontiguousarray(
            np.stack([ep_h, ep_t])).astype(bf16)         # [2, E, H]
        epo = np.ascontiguousarray(
            np.stack([ep_h.T, ep_t.T])).astype(bf16)     # [2, H, E]
        ptT = np.ascontiguousarray(
            pro[b].reshape(RP, 2 * H).T).astype(bf16)
        in_maps.append({
            "at": at,
            "seq": seq[b, q].astype(bf16),
            "wh": wh,
            "wt": wt,
            "ptT": ptT,
            "epT": epT,
            "epo": epo,
            "m16": m16,
            "m32": m32,
        })
    return in_maps


def kernel(sequence_output, attention, W_head, W_tail, prototypes,
           mention_pos):
    from concourse.bass_utils import run_bass_kernel_spmd

    if "nc" not in _CACHE:
        _CACHE["nc"] = _build_program()
    nc = _CACHE["nc"]

    in_maps = _host_prep(sequence_output, attention, W_head, W_tail,
                         prototypes, mention_pos)
    res = run_bass_kernel_spmd(nc, in_maps, core_ids=list(range(NCORES)))

    out = np.empty((B, Q, E, E, R), dtype=np.float32)
    for c in range(NCORES):
        b, q = divmod(c, Q)
        out[b, q] = res.results[c]["out"].reshape(E, E, R)
    return out
